# revision 33
# baseline (speedup 1.0000x reference)
"""DotAttackHead kernel for Trainium2 (8 NeuronCores, data-parallel over batch).

prob = softmax(relu(ufeat @ W.T + b) @ efeat.T / sqrt(256) + mask_bias)
W = g * v / ||v||_F

Sharding: batch 64 -> 8 cores x 8 batches. Params replicated.

Host prep: weight-norm W, transpose+bf16-cast of ufeat/efeat (PE needs the
contraction dim on partitions), mask -> additive bias {0, -16e9} (pre-scaled
by 16 so the 1/sqrt(256) division can fuse into the Exp activation's scale).

Device per batch b:
  mm1:  projT[e,u] = relu(wT.T @ ufT[b] + bias)    (PE bf16 + DVE add/max)
  mm2:  psum[u,n]  = projT.T @ efT[b] + ones.T @ maskbias16[b]   (PE bf16)
  soft: e = Exp(psum/16) with accum_out row-sum (ACT), r = 1/s (DVE),
        prob = e * r (DVE), DMA out f32.
No max-subtraction: logits are O(+-6) so exp is safe in fp32, and softmax is
shift-invariant so this matches the reference; masked lanes get -1e9 and
underflow to exactly 0 like the reference does.
"""

from contextlib import ExitStack

import ml_dtypes
import numpy as np

import concourse.bass as bass
import concourse.mybir as mybir
import concourse.tile as tile
from concourse import bacc
from concourse.bass_utils import run_bass_kernel_spmd

N_CORES = 8
B = 64
U = 1024  # units
E = 256   # efeat dim
K = 512   # ufeat dim
N = 1024  # enemies
BPC = B // N_CORES  # batches per core

F32 = mybir.dt.float32
BF16 = mybir.dt.bfloat16
BF16_NP = ml_dtypes.bfloat16

def _build_bass(bpc: int = BPC) -> bass.Bass:
    # Bacc (not raw Bass): its finalize() runs generate_event_semaphores,
    # which splits multi-wait instructions to satisfy TRN2's 1-wait limit.
    nc = bacc.Bacc(None, target_bir_lowering=False)

    ufT = nc.declare_dram_parameter("ufT", [bpc, K, U], BF16, isOutput=False)
    efT = nc.declare_dram_parameter("efT", [bpc, E, N], BF16, isOutput=False)
    wT = nc.declare_dram_parameter("wT", [K, E], BF16, isOutput=False)
    bias = nc.declare_dram_parameter("bias", [E], F32, isOutput=False)
    # bf16 output store halves the dominant DMA stream; host upcasts to f32.
    prob = nc.declare_dram_parameter("prob", [bpc, U, N], BF16, isOutput=True)

    with tile.TileContext(nc) as tc, ExitStack() as ctx:
        singles = ctx.enter_context(tc.tile_pool(name="singles", bufs=1))
        pin = ctx.enter_context(tc.tile_pool(name="pin", bufs=5))
        pproj = ctx.enter_context(tc.tile_pool(name="pproj", bufs=3))
        pet = ctx.enter_context(tc.tile_pool(name="pet", bufs=8))
        pprob = ctx.enter_context(tc.tile_pool(name="pprob", bufs=8))
        psmall = ctx.enter_context(tc.tile_pool(name="psmall", bufs=16))
        pps1 = ctx.enter_context(tc.tile_pool(name="pps1", bufs=2, space="PSUM"))
        pps2 = ctx.enter_context(tc.tile_pool(name="pps2", bufs=3, space="PSUM"))

        # ---- resident constants ----
        # wT as 4 k-tiles: wt_sb[p, kt, e] = wT[kt*128+p, e]
        wt_sb = singles.tile([128, 4, E], BF16)
        nc.sync.dma_start(out=wt_sb, in_=wT[:, :].rearrange("(kt p) e -> p kt e", p=128))
        # bias as 2 e-tiles on partitions: b_sb[p, et] = bias[et*128+p]
        b_sb = singles.tile([128, 2], F32)
        nc.sync.dma_start(out=b_sb, in_=bias[:].rearrange("(et p) -> p et", p=128))

        def emit_loads(bi):
            uft = pin.tile([128, 4, U], BF16, tag="uft")
            nc.sync.dma_start(
                out=uft, in_=ufT[bi, :, :].rearrange("(kt p) u -> p kt u", p=128)
            )
            eft = pin.tile([128, 2, N], BF16, tag="eft")
            nc.sync.dma_start(
                out=eft, in_=efT[bi, :, :].rearrange("(et p) n -> p et n", p=128)
            )
            return uft, eft

        def emit_mm1_group(uft, projT, gi):
            # group gi -> (ej, uc): projT[e,u] = relu(wT.T @ ufT + b), bf16 out
            ej, uc = gi // 2, gi % 2
            esl = slice(ej * 128, (ej + 1) * 128)
            usl = slice(uc * 512, (uc + 1) * 512)
            ps1 = pps1.tile([128, 512], F32, tag="ps1")
            for kj in range(4):
                nc.tensor.matmul(
                    ps1,
                    lhsT=wt_sb[:, kj, esl],
                    rhs=uft[:, kj, usl],
                    start=(kj == 0),
                    stop=(kj == 3),
                )
            # relu(x + b) = max(x + b, 0) fused on DVE; casts to bf16
            nc.vector.tensor_scalar(
                out=projT[:, ej, usl],
                in0=ps1,
                scalar1=b_sb[:, ej : ej + 1],
                scalar2=0.0,
                op0=mybir.AluOpType.add,
                op1=mybir.AluOpType.max,
            )

        def emit_softmax_tile(bi, projT, eft, ui):
            uslice = slice(ui * 128, (ui + 1) * 128)
            ps2 = pps2.tile([128, N], F32, tag="ps2")
            # e-major: consecutive matmuls share the same lhsT (weight reuse)
            for ej in range(2):
                for nck in range(2):
                    nsl = slice(nck * 512, (nck + 1) * 512)
                    nc.tensor.matmul(
                        ps2[:, nsl],
                        lhsT=projT[:, ej, uslice],
                        rhs=eft[:, ej, nsl],
                        start=(ej == 0),
                        stop=(ej == 1),
                    )
            et = pet.tile([128, N], BF16, tag="et")
            s = psmall.tile([128, 1], F32, tag="s")
            nc.scalar.activation(
                out=et,
                in_=ps2,
                func=mybir.ActivationFunctionType.Exp,
                scale=1.0 / 16.0,
                accum_out=s,
            )
            r = psmall.tile([128, 1], F32, tag="r")
            nc.vector.reciprocal(out=r, in_=s)
            prob_t = pprob.tile([128, N], BF16, tag="prob")
            nc.vector.tensor_scalar_mul(out=prob_t, in0=et, scalar1=r)
            nc.sync.dma_start(out=prob[bi, uslice, :], in_=prob_t)

        # Software-pipelined emission: mm1 groups for batch bi+1 are emitted
        # between softmax tiles of batch bi's second half, so the PE never
        # monopolizes a contiguous ~4us window on mm1 while ACT's 3-deep
        # PSUM backlog drains.
        tiles = {0: emit_loads(0)}
        projs = {0: pproj.tile([128, 2, U], BF16, tag="projT", name="projT0")}
        for gi in range(4):
            emit_mm1_group(tiles[0][0], projs[0], gi)
        for bi in range(bpc):
            uft, eft = tiles[bi]
            projT = projs[bi]
            if bi + 1 < bpc:
                tiles[bi + 1] = emit_loads(bi + 1)
            for ui in range(4):
                emit_softmax_tile(bi, projT, eft, ui)
            if bi + 1 < bpc:
                projs[bi + 1] = pproj.tile(
                    [128, 2, U], BF16, tag="projT", name=f"projT{bi + 1}"
                )
            # mm1 groups for bi+1 ride along u4..u7 so the PE never
            # monopolizes a contiguous ~4us window on mm1 while ACT's
            # 3-deep PSUM backlog drains
            for ui in range(4, 8):
                emit_softmax_tile(bi, projT, eft, ui)
                if bi + 1 < bpc:
                    emit_mm1_group(tiles[bi + 1][0], projs[bi + 1], ui - 4)

    # Runs Bacc.compile(): register allocation + event-semaphore splitting.
    nc.finalize()
    return nc


def _prep_inputs(ufeat, efeat, num_enemy, v, g, b):
    """Host-side prep: weight-norm, transpose + bf16 cast, mask bias."""
    ufeat = np.asarray(ufeat, dtype=np.float32)
    efeat = np.asarray(efeat, dtype=np.float32)
    num_enemy = np.asarray(num_enemy).astype(np.int64)
    v = np.asarray(v, dtype=np.float32)
    g = np.float32(np.asarray(g))
    b = np.asarray(b, dtype=np.float32)

    W = (g / np.float32(np.linalg.norm(v))) * v  # [E, K]
    wT = np.ascontiguousarray(W.T).astype(BF16_NP)  # [K, E]

    # [B, K, U] / [B, E, N] bf16 (cast first: halves the transpose traffic)
    ufT = ufeat.astype(BF16_NP).transpose(0, 2, 1)
    efT = np.ascontiguousarray(efeat.astype(BF16_NP).transpose(0, 2, 1))

    # Mask: poison masked efeat columns (n >= num_enemy) with -1e30. Since
    # proj >= 0 (relu) and a proj row is never identically 0 in practice,
    # masked logits land at <= -1e28 and exp underflows to exactly 0 — the
    # same 0 the reference's -1e9 bias produces. num_enemy==0 => all lanes
    # masked => the reference's uniform -1e9 shift cancels in softmax =>
    # leave those batches unpoisoned.
    ne = np.where(num_enemy > 0, num_enemy, N)
    col_masked = np.arange(N)[None, :] >= ne[:, None]  # [B, N]
    efT[np.broadcast_to(col_masked[:, None, :], efT.shape)] = BF16_NP(-1e30)

    return ufT, efT, wT, b


_nc_cache: dict[int, bass.Bass] = {}


def run(ufeat, efeat, num_enemy, v, g, b, trace=False):
    ufT, efT, wT, b = _prep_inputs(ufeat, efeat, num_enemy, v, g, b)

    if BPC not in _nc_cache:
        _nc_cache[BPC] = _build_bass(BPC)
    nc = _nc_cache[BPC]

    in_maps = []
    for c in range(N_CORES):
        sl = slice(c * BPC, (c + 1) * BPC)
        in_maps.append({"ufT": ufT[sl], "efT": efT[sl], "wT": wT, "bias": b})

    res = run_bass_kernel_spmd(nc, in_maps, list(range(N_CORES)), trace=trace)
    out = np.concatenate(
        [res.results[c]["prob"].astype(np.float32) for c in range(N_CORES)], axis=0
    )
    return out, res


def kernel(ufeat, efeat, num_enemy, v, g, b):
    out, _ = run(ufeat, efeat, num_enemy, v, g, b, trace=False)
    return out
